# revision 31
# baseline (speedup 1.0000x reference)
"""Trainium2 Bass kernel for nn_MLP_Interpolate.

Reference computation (out_size=512, H=W=128 -> exact 4x nearest upsample):
  out[b, :, 4k+r, 4l+s] = relu(x[b,:,k,l] @ W1[:64] + c[r,s]) @ W2 + b2
  c[r,s] = rel_y(r)*W1[64] + rel_x(s)*W1[65] + b1,  rel(t) = (2t-3)/4

Strategy (8 cores, shard = (batch, H-half); all device math fp16 except
fp32 PSUM accumulation and fp32 output):
  - F = W1c^T x on PE with a 128x128 block-diagonal stationary so two
    64-channel pixel groups share each streamed column.
  - F copied PSUM->SBUF as fp16 (DVE cast), then all 16 bias+relu
    variants on DVE fp16 tensor_scalar (hits the 4X perf mode).
  - pred = h @ W2 on PE with a [128,32] zero-padded block-diag
    stationary, packed 3 matmuls per PSUM tile at column-tile positions
    0/32/64; j-dim = r so each partition line holds 4 consecutive
    output rows.  Stationary columns are ordered c-major (k = 2c+g) so
    one output DMA covers both pixel groups with a [3,2,2048] AP.
  - ACT evacuates pred PSUM->SBUF; 8 DMAs per tile stream [6,2048]
    chunks (48KB) straight to DRAM, rotated across queues.
"""

import numpy as np

import concourse.bass as bass
import concourse.bacc as bacc
import concourse.mybir as mybir
import concourse.tile as tile
from concourse.bass_utils import run_bass_kernel_spmd

# Problem constants (hardcoded per contract)
B, C, H, W = 4, 64, 128, 128
OUT = 512
NF = 64
N_CORES = 8
ROWS_PER_CORE = H // 2          # 64 input rows per core
NT = 4                          # tiles of 16 input rows (8 per group)
REL = np.array([-0.75, -0.25, 0.25, 0.75], dtype=np.float32)

_CACHE = {}


def _build_program():
    if "nc" in _CACHE:
        return _CACHE["nc"]

    fp32 = mybir.dt.float32
    fp16 = mybir.dt.float16
    nc = bacc.Bacc("TRN2", target_bir_lowering=False, debug=False,
                   num_devices=N_CORES)

    # x packed on host: [part = c + 64g, t, i*128 + l] where input row
    # rho = 16t + 8g + i
    x_d = nc.dram_tensor("x", [128, NT, 8 * W], fp16, kind="ExternalInput")
    w1_d = nc.dram_tensor("w1diag", [128, 128], fp16, kind="ExternalInput")
    w2_d = nc.dram_tensor("w2diag", [128, 32], fp16, kind="ExternalInput")
    crs_d = nc.dram_tensor("crsT", [128, 16], fp32, kind="ExternalInput")
    # out row = 64t + 32g + 4i + r, laid out [k=2c+g, t, (4i+r)*512 + l]
    # (dim0 matches the stationary's c-major column order, so output DMAs
    # need no separate g dimension)
    out_d = nc.dram_tensor("out", [6, NT, 32 * OUT], fp32,
                           kind="ExternalOutput")

    with tile.TileContext(nc) as tc:
        with (
            tc.tile_pool(name="consts", bufs=1) as consts,
            tc.tile_pool(name="xbuf", bufs=1) as xbuf,
            tc.tile_pool(name="fbuf", bufs=2) as fbuf,
            tc.tile_pool(name="hbuf", bufs=2) as hbuf,
            tc.tile_pool(name="stbuf", bufs=2) as stbuf,
            tc.tile_pool(name="fpsum", bufs=1, space=bass.MemorySpace.PSUM) as fpsum,
            tc.tile_pool(name="ppsum", bufs=3, space=bass.MemorySpace.PSUM) as ppsum,
        ):
            w1_sb = consts.tile([128, 128], fp16)
            w2_sb = consts.tile([128, 32], fp16)
            crs_sb = consts.tile([128, 16], fp32)
            xall = xbuf.tile([128, NT, 8 * W], fp16)
            nc.sync.dma_start(xall[:, 0, :], x_d[:, 0, :])
            nc.scalar.dma_start(w1_sb[:], w1_d[:])
            nc.scalar.dma_start(w2_sb[:], w2_d[:])
            nc.scalar.dma_start(crs_sb[:], crs_d[:])
            nc.sync.dma_start(xall[:, 1:NT, :], x_d[:, 1:NT, :])

            f_tiles = [None] * NT
            h_tiles = [None] * NT
            st_tiles = {}

            def feat_matmul(t, reps=1):
                ft = fpsum.tile([128, 8, W], fp32, tag="ft")
                for _ in range(reps):
                    for h in range(2):
                        nc.tensor.matmul(
                            ft[:, 4 * h:4 * h + 4, :], w1_sb[:],
                            xall[:, t, 512 * h:512 * h + 512],
                            start=True, stop=True)
                f_tiles[t] = ft

            def relus(t):
                # fb: fp16 copy of F so DVE tensor_scalar hits 4X mode
                fb = fbuf.tile([128, 8, W], fp16, tag="fb")
                nc.vector.tensor_copy(fb[:, :, :], f_tiles[t][:, :, :])
                hr = hbuf.tile([128, 16, 8, W], fp16, tag="hr")
                # At t=0 ACT is otherwise idle (no pred copies yet), so
                # giving it the tail variants shortens the prologue.
                act_v = set(range(11, 16)) if t == 0 else ()
                for v in range(16):
                    if v in act_v:
                        nc.scalar.activation(
                            hr[:, v, :, :], fb[:, :, :],
                            mybir.ActivationFunctionType.Relu,
                            bias=crs_sb[:, v:v + 1])
                    else:
                        nc.vector.tensor_scalar(
                            hr[:, v, :, :], fb[:, :, :],
                            crs_sb[:, v:v + 1], 0.0,
                            mybir.AluOpType.add, mybir.AluOpType.max)
                h_tiles[t] = hr

            def dmas_for_m(t, m, engines, n0, rh=None, pair=False):
                # one DMA per q covering both groups; rh=None sends all
                # four r rows of input row i = 3q + m, rh=0/1 sends just
                # that r-pair, pair=True sends both tiles of a t-pair
                nq = 3 if m < 2 else 2
                st = st_tiles[(t, m)]
                n = n0
                for q in range(nq):
                    i = 3 * q + m
                    off = 4 * i * OUT
                    if pair:
                        src = st[32 * q:32 * q + 6, :, :, :] \
                            .rearrange("p t r l -> p t (r l)")
                        dst = out_d[:, t:t + 2, off:off + 4 * OUT]
                    elif rh is None:
                        src = st[32 * q:32 * q + 6, :, :] \
                            .rearrange("p r l -> p (r l)")
                        dst = out_d[:, t, off:off + 4 * OUT]
                    else:
                        src = st[32 * q:32 * q + 6, 2 * rh:2 * rh + 2, :] \
                            .rearrange("p r l -> p (r l)")
                        off = (4 * i + 2 * rh) * OUT
                        dst = out_d[:, t, off:off + 2 * OUT]
                    engines[n % len(engines)].dma_start(dst, src)
                    n += 1
                return n

            def pred(t):
                hr = h_tiles[t]
                # t0/t1 stage into one pair tile (DMA'd together during
                # t2's window); t2/t3 stage per-tile
                if t == 0:
                    for m in range(3):
                        np_ = 70 if m < 2 else 38
                        stt = stbuf.tile([np_, 2, 4, OUT], fp32, bufs=1,
                                         tag=f"stp{m}", name=f"stp{m}")
                        st_tiles[(0, m)] = st_tiles[(1, m)] = stt
                elif t >= 2:
                    for m in range(3):
                        np_ = 70 if m < 2 else 38
                        stt = stbuf.tile([np_, 4, OUT], fp32,
                                         tag=f"st{m}", name=f"st{m}")
                        st_tiles[(t, m)] = stt
                ndma = 0
                # m-major: each m's output DMAs can start as soon as its
                # two pred tiles are evacuated
                for m in range(3):
                    nq = 3 if m < 2 else 2
                    np_ = 70 if m < 2 else 38
                    for rh in range(2):
                        pt = ppsum.tile([96, 2, OUT], fp32, tag="pt")
                        for q in range(nq):
                            i = 3 * q + m
                            for j in range(2):
                                r = 2 * rh + j
                                # rhs streamed s-major with l contiguous
                                # (fast PE streaming); the host gather
                                # un-permutes columns from (s,l) to 4l+s
                                rhs = hr[:, 4 * r:4 * r + 4, i, :]
                                nc.tensor.matmul(
                                    pt[32 * q:32 * q + 32, j, :],
                                    w2_sb[:], rhs, start=True, stop=True)
                        # m2 copies at t=3 go to DVE (idle by then) so the
                        # final DMAs can start sooner
                        if t < 2:
                            st_dst = st_tiles[(t, m)][:, t, 2 * rh:2 * rh + 2, :]
                        else:
                            st_dst = st_tiles[(t, m)][:, 2 * rh:2 * rh + 2, :]
                        if t == NT - 1 and m == 2:
                            nc.vector.tensor_copy(st_dst, pt[0:np_, :, :])
                        else:
                            nc.scalar.activation(
                                st_dst, pt[0:np_, :, :],
                                mybir.ActivationFunctionType.Copy)
                        if t == NT - 1:
                            # last tile: stream each rh half out as soon
                            # as it is evacuated, so almost all output is
                            # in flight when the final copy lands; keep
                            # sync light (its ring is backlogged with
                            # t2's transfers)
                            fin = [nc.gpsimd, nc.scalar, nc.gpsimd, nc.sync]
                            ndma = dmas_for_m(t, m, fin, ndma, rh=rh)
                    # Mid-run DMAs stay off gpsimd: its SWDGE descriptor
                    # writes share the DVE SBUF port and slow the relu
                    # tensor_scalars.  By t>=2's issue window DVE is done,
                    # so gpsimd is safe to use there.
                    if t == 1:
                        ndma = dmas_for_m(0, m, [nc.sync], ndma, pair=True)
                    elif t == 2:
                        ndma = dmas_for_m(t, m, [nc.gpsimd, nc.sync], ndma)

            feat_matmul(0)
            for t in range(NT):
                relus(t)
                if t + 1 < NT:
                    # extra reps keep the PE HAM-warm through the relu
                    # windows (PE would otherwise micro-idle and throttle
                    # down to 1.2 GHz, slowing every subsequent matmul)
                    feat_matmul(t + 1, reps=6 if t == 0 else 2)
                pred(t)

    nc.compile()
    _CACHE["nc"] = nc
    return nc


def _prep_inputs(x, W1, b1, W2, b2):
    x = np.asarray(x, dtype=np.float32)
    W1 = np.asarray(W1, dtype=np.float32)
    b1 = np.asarray(b1, dtype=np.float32)
    W2 = np.asarray(W2, dtype=np.float32)

    w1c = W1[:NF]                      # [64, 64]
    w1diag = np.zeros((128, 128), dtype=np.float16)
    w1diag[0:64, 0:64] = w1c
    w1diag[64:128, 64:128] = w1c

    # stationary columns k = 2c + g (c-major) so the output DMA's
    # partition iteration matches a [3, 2, 2048] DRAM AP
    w2diag = np.zeros((128, 32), dtype=np.float16)
    for g in range(2):
        for ch in range(3):
            w2diag[64 * g:64 * g + 64, 2 * ch + g] = W2[:, ch]

    # c[v=4r+s, phi] = rel[r]*W1[64] + rel[s]*W1[65] + b1 -> [16, 64]
    crs = (REL[:, None, None] * W1[NF][None, None, :]
           + REL[None, :, None] * W1[NF + 1][None, None, :]
           + b1[None, None, :]).reshape(16, NF)
    crsT = np.ascontiguousarray(
        np.concatenate([crs.T, crs.T], axis=0)).astype(np.float32)  # [128,16]

    in_maps = []
    for c in range(N_CORES):
        b, half = c // 2, c % 2
        xs = x[b, :, half * ROWS_PER_CORE:(half + 1) * ROWS_PER_CORE, :]
        # [c, rho, l] -> [c, t, g, i, l] -> [g, c, t, i, l] -> [128, t, i*l]
        xp = np.ascontiguousarray(
            xs.reshape(NF, NT, 2, 8, W).transpose(2, 0, 1, 3, 4)
            .reshape(128, NT, 8 * W).astype(np.float16))
        in_maps.append({"x": xp, "w1diag": w1diag, "w2diag": w2diag,
                        "crsT": crsT})
    return in_maps


def _gather(results, b2):
    full = np.empty((B, 3, OUT, OUT), dtype=np.float32)
    for c in range(N_CORES):
        b, half = c // 2, c % 2
        # [k=2c+g, t, 32*512] -> rows ordered (t, g, 4i+r); columns come
        # back (s, l)-ordered, un-permute to 4l+s here
        o = (results[c]["out"].reshape(3, 2, NT, 32, 4, W)
             .transpose(0, 2, 1, 3, 5, 4).reshape(3, 4 * ROWS_PER_CORE, OUT))
        full[b, :, half * (OUT // 2):(half + 1) * (OUT // 2), :] = o
    b2 = np.asarray(b2, dtype=np.float32)
    if np.any(b2):
        full += b2.reshape(1, 3, 1, 1)
    return full


def run(trace=False, **inputs):
    nc = _build_program()
    in_maps = _prep_inputs(inputs["x"], inputs["W1"], inputs["b1"],
                           inputs["W2"], inputs["b2"])
    res = run_bass_kernel_spmd(nc, in_maps, list(range(N_CORES)), trace=trace)
    return _gather(res.results, inputs["b2"]), res


def kernel(**inputs):
    out, _ = run(trace=False, **inputs)
    return out


# revision 32
# speedup vs baseline: 1.0780x; 1.0780x over previous
"""Trainium2 Bass kernel for nn_MLP_Interpolate.

Reference computation (out_size=512, H=W=128 -> exact 4x nearest upsample):
  out[b, :, 4k+r, 4l+s] = relu(x[b,:,k,l] @ W1[:64] + c[r,s]) @ W2 + b2
  c[r,s] = rel_y(r)*W1[64] + rel_x(s)*W1[65] + b1,  rel(t) = (2t-3)/4

Strategy (8 cores, shard = (batch, H-half); all device math fp16 except
fp32 PSUM accumulation and fp32 output):
  - F = W1c^T x on PE with a 128x128 block-diagonal stationary so two
    64-channel pixel groups share each streamed column.
  - F copied PSUM->SBUF as fp16 (DVE cast), then all 16 bias+relu
    variants on DVE fp16 tensor_scalar (hits the 4X perf mode).
  - pred = h @ W2 on PE with a [128,32] zero-padded block-diag
    stationary, packed 3 matmuls per PSUM tile at column-tile positions
    0/32/64; j-dim = r so each partition line holds 4 consecutive
    output rows.  Stationary columns are ordered c-major (k = 2c+g) so
    one output DMA covers both pixel groups with a [3,2,2048] AP.
  - ACT evacuates pred PSUM->SBUF; 8 DMAs per tile stream [6,2048]
    chunks (48KB) straight to DRAM, rotated across queues.
"""

import numpy as np

import concourse.bass as bass
import concourse.bacc as bacc
import concourse.mybir as mybir
import concourse.tile as tile
from concourse.bass_utils import run_bass_kernel_spmd

# Problem constants (hardcoded per contract)
B, C, H, W = 4, 64, 128, 128
OUT = 512
NF = 64
N_CORES = 8
ROWS_PER_CORE = H // 2          # 64 input rows per core
NT = 4                          # tiles of 16 input rows (8 per group)
REL = np.array([-0.75, -0.25, 0.25, 0.75], dtype=np.float32)

_CACHE = {}


def _build_program():
    if "nc" in _CACHE:
        return _CACHE["nc"]

    fp32 = mybir.dt.float32
    fp16 = mybir.dt.float16
    nc = bacc.Bacc("TRN2", target_bir_lowering=False, debug=False,
                   num_devices=N_CORES)

    # x packed on host: [part = c + 64g, t, i*128 + l] where input row
    # rho = 16t + 8g + i
    x_d = nc.dram_tensor("x", [128, NT, 8 * W], fp16, kind="ExternalInput")
    w1_d = nc.dram_tensor("w1diag", [128, 128], fp16, kind="ExternalInput")
    w2_d = nc.dram_tensor("w2diag", [128, 32], fp16, kind="ExternalInput")
    crs_d = nc.dram_tensor("crsT", [128, 16], fp32, kind="ExternalInput")
    # out row = 64t + 32g + 4i + r, laid out [k=2c+g, t, (4i+r)*512 + l]
    # (dim0 matches the stationary's c-major column order, so output DMAs
    # need no separate g dimension)
    out_d = nc.dram_tensor("out", [6, NT, 32 * OUT], fp32,
                           kind="ExternalOutput")

    with tile.TileContext(nc) as tc:
        with (
            tc.tile_pool(name="consts", bufs=1) as consts,
            tc.tile_pool(name="xbuf", bufs=1) as xbuf,
            tc.tile_pool(name="fbuf", bufs=2) as fbuf,
            tc.tile_pool(name="hbuf", bufs=2) as hbuf,
            tc.tile_pool(name="stbuf", bufs=2) as stbuf,
            tc.tile_pool(name="fpsum", bufs=1, space=bass.MemorySpace.PSUM) as fpsum,
            tc.tile_pool(name="ppsum", bufs=3, space=bass.MemorySpace.PSUM) as ppsum,
        ):
            w1_sb = consts.tile([128, 128], fp16)
            w2_sb = consts.tile([128, 32], fp16)
            crs_sb = consts.tile([128, 16], fp32)
            xall = xbuf.tile([128, NT, 8 * W], fp16)
            nc.sync.dma_start(xall[:, 0, :], x_d[:, 0, :])
            nc.scalar.dma_start(w1_sb[:], w1_d[:])
            nc.scalar.dma_start(w2_sb[:], w2_d[:])
            nc.scalar.dma_start(crs_sb[:], crs_d[:])
            nc.sync.dma_start(xall[:, 1:NT, :], x_d[:, 1:NT, :])

            f_tiles = [None] * NT
            h_tiles = [None] * NT
            st_tiles = {}

            def feat_matmul(t, reps=1):
                ft = fpsum.tile([128, 8, W], fp32, tag="ft")
                for _ in range(reps):
                    for h in range(2):
                        nc.tensor.matmul(
                            ft[:, 4 * h:4 * h + 4, :], w1_sb[:],
                            xall[:, t, 512 * h:512 * h + 512],
                            start=True, stop=True)
                f_tiles[t] = ft

            def relus(t):
                # fb: fp16 copy of F so DVE tensor_scalar hits 4X mode
                fb = fbuf.tile([128, 8, W], fp16, tag="fb")
                nc.vector.tensor_copy(fb[:, :, :], f_tiles[t][:, :, :])
                hr = hbuf.tile([128, 16, 8, W], fp16, tag="hr")
                # At t=0 ACT is otherwise idle (no pred copies yet), so
                # giving it the tail variants shortens the prologue.
                act_v = set(range(11, 16)) if t == 0 else ()
                for v in range(16):
                    if v in act_v:
                        nc.scalar.activation(
                            hr[:, v, :, :], fb[:, :, :],
                            mybir.ActivationFunctionType.Relu,
                            bias=crs_sb[:, v:v + 1])
                    else:
                        nc.vector.tensor_scalar(
                            hr[:, v, :, :], fb[:, :, :],
                            crs_sb[:, v:v + 1], 0.0,
                            mybir.AluOpType.add, mybir.AluOpType.max)
                h_tiles[t] = hr

            def dmas_for_m(t, m, engines, n0, rh=None, pair=False):
                # one DMA per q covering both groups; rh=None sends all
                # four r rows of input row i = 3q + m, rh=0/1 sends just
                # that r-pair, pair=True sends both tiles of a t-pair
                nq = 3 if m < 2 else 2
                st = st_tiles[(t, m)]
                n = n0
                for q in range(nq):
                    i = 3 * q + m
                    off = 4 * i * OUT
                    if pair:
                        src = st[32 * q:32 * q + 6, :, :, :] \
                            .rearrange("p t r l -> p t (r l)")
                        dst = out_d[:, t:t + 2, off:off + 4 * OUT]
                    elif rh is None:
                        src = st[32 * q:32 * q + 6, :, :] \
                            .rearrange("p r l -> p (r l)")
                        dst = out_d[:, t, off:off + 4 * OUT]
                    else:
                        src = st[32 * q:32 * q + 6, 2 * rh:2 * rh + 2, :] \
                            .rearrange("p r l -> p (r l)")
                        off = (4 * i + 2 * rh) * OUT
                        dst = out_d[:, t, off:off + 2 * OUT]
                    engines[n % len(engines)].dma_start(dst, src)
                    n += 1
                return n

            def pred(t):
                hr = h_tiles[t]
                for m in range(3):
                    np_ = 70 if m < 2 else 38
                    stt = stbuf.tile([np_, 4, OUT], fp32,
                                     tag=f"st{m}", name=f"st{m}")
                    st_tiles[(t, m)] = stt
                ndma = 0
                # m-major: each m's output DMAs can start as soon as its
                # two pred tiles are evacuated
                for m in range(3):
                    nq = 3 if m < 2 else 2
                    np_ = 70 if m < 2 else 38
                    for rh in range(2):
                        pt = ppsum.tile([96, 2, OUT], fp32, tag="pt")
                        for q in range(nq):
                            i = 3 * q + m
                            for j in range(2):
                                r = 2 * rh + j
                                # rhs streamed s-major with l contiguous
                                # (fast PE streaming); the host gather
                                # un-permutes columns from (s,l) to 4l+s
                                rhs = hr[:, 4 * r:4 * r + 4, i, :]
                                nc.tensor.matmul(
                                    pt[32 * q:32 * q + 32, j, :],
                                    w2_sb[:], rhs, start=True, stop=True)
                        # m2 copies at t=3 go to DVE (idle by then) so the
                        # final DMAs can start sooner
                        st_dst = st_tiles[(t, m)][:, 2 * rh:2 * rh + 2, :]
                        if t == NT - 1 and m == 2:
                            nc.vector.tensor_copy(st_dst, pt[0:np_, :, :])
                        else:
                            nc.scalar.activation(
                                st_dst, pt[0:np_, :, :],
                                mybir.ActivationFunctionType.Copy)
                        if t == NT - 1:
                            # last tile: stream each rh half out as soon
                            # as it is evacuated, so almost all output is
                            # in flight when the final copy lands; keep
                            # sync light (its ring is backlogged with
                            # t2's transfers)
                            fin = [nc.gpsimd, nc.scalar, nc.gpsimd, nc.sync]
                            ndma = dmas_for_m(t, m, fin, ndma, rh=rh)
                    # Mid-run DMAs stay off gpsimd: its SWDGE descriptor
                    # writes share the DVE SBUF port and slow the relu
                    # tensor_scalars.  By t>=2's issue window DVE is done,
                    # so gpsimd is safe to use there.
                    if t < 2:
                        ndma = dmas_for_m(t, m, [nc.sync], ndma)
                    elif t == 2:
                        ndma = dmas_for_m(t, m, [nc.gpsimd, nc.sync], ndma)

            feat_matmul(0)
            for t in range(NT):
                relus(t)
                if t + 1 < NT:
                    # extra reps keep the PE HAM-warm through the relu
                    # windows (PE would otherwise micro-idle and throttle
                    # down to 1.2 GHz, slowing every subsequent matmul)
                    feat_matmul(t + 1, reps=6 if t == 0 else 2)
                pred(t)

    nc.compile()
    _CACHE["nc"] = nc
    return nc


def _prep_inputs(x, W1, b1, W2, b2):
    x = np.asarray(x, dtype=np.float32)
    W1 = np.asarray(W1, dtype=np.float32)
    b1 = np.asarray(b1, dtype=np.float32)
    W2 = np.asarray(W2, dtype=np.float32)

    w1c = W1[:NF]                      # [64, 64]
    w1diag = np.zeros((128, 128), dtype=np.float16)
    w1diag[0:64, 0:64] = w1c
    w1diag[64:128, 64:128] = w1c

    # stationary columns k = 2c + g (c-major) so the output DMA's
    # partition iteration matches a [3, 2, 2048] DRAM AP
    w2diag = np.zeros((128, 32), dtype=np.float16)
    for g in range(2):
        for ch in range(3):
            w2diag[64 * g:64 * g + 64, 2 * ch + g] = W2[:, ch]

    # c[v=4r+s, phi] = rel[r]*W1[64] + rel[s]*W1[65] + b1 -> [16, 64]
    crs = (REL[:, None, None] * W1[NF][None, None, :]
           + REL[None, :, None] * W1[NF + 1][None, None, :]
           + b1[None, None, :]).reshape(16, NF)
    crsT = np.ascontiguousarray(
        np.concatenate([crs.T, crs.T], axis=0)).astype(np.float32)  # [128,16]

    in_maps = []
    for c in range(N_CORES):
        b, half = c // 2, c % 2
        xs = x[b, :, half * ROWS_PER_CORE:(half + 1) * ROWS_PER_CORE, :]
        # [c, rho, l] -> [c, t, g, i, l] -> [g, c, t, i, l] -> [128, t, i*l]
        xp = np.ascontiguousarray(
            xs.reshape(NF, NT, 2, 8, W).transpose(2, 0, 1, 3, 4)
            .reshape(128, NT, 8 * W).astype(np.float16))
        in_maps.append({"x": xp, "w1diag": w1diag, "w2diag": w2diag,
                        "crsT": crsT})
    return in_maps


def _gather(results, b2):
    full = np.empty((B, 3, OUT, OUT), dtype=np.float32)
    for c in range(N_CORES):
        b, half = c // 2, c % 2
        # [k=2c+g, t, 32*512] -> rows ordered (t, g, 4i+r); columns come
        # back (s, l)-ordered, un-permute to 4l+s here
        o = (results[c]["out"].reshape(3, 2, NT, 32, 4, W)
             .transpose(0, 2, 1, 3, 5, 4).reshape(3, 4 * ROWS_PER_CORE, OUT))
        full[b, :, half * (OUT // 2):(half + 1) * (OUT // 2), :] = o
    b2 = np.asarray(b2, dtype=np.float32)
    if np.any(b2):
        full += b2.reshape(1, 3, 1, 1)
    return full


def run(trace=False, **inputs):
    nc = _build_program()
    in_maps = _prep_inputs(inputs["x"], inputs["W1"], inputs["b1"],
                           inputs["W2"], inputs["b2"])
    res = run_bass_kernel_spmd(nc, in_maps, list(range(N_CORES)), trace=trace)
    return _gather(res.results, inputs["b2"]), res


def kernel(**inputs):
    out, _ = run(trace=False, **inputs)
    return out
